# revision 31
# baseline (speedup 1.0000x reference)
"""MoE (top-2 of 8 experts) Trainium2 kernel, 8-core slot-parallel.

Strategy
--------
The reference output depends only on each token's top-2 experts, so the
device computes the *sparse* FFN: 8192 tokens x 2 = 16384 (token, expert)
slots, split evenly across 8 cores (2048 slots each, +pad).

Host side (all O(N*D) data movement or O(N*E) router math; the O(N*D*H)
FLOPs all stay on device):
  - f64 router (logits -> top-2 + softmax gates), identical selection rule
    to jax.lax.top_k (stable order).
  - greedy balanced token->core assignment so that every expert's
    per-core count is nearly equal -> per-expert capacity padding is ~0.3%.
  - gathers + transposes the selected tokens into a per-core xselT tensor
    (f16), so the device needs no router, no one-hot gathers, no scatter.
  - after the run: adds b2, applies the gates and combines the two slots
    of each token (2 FLOPs/element), and unshards.

Device program per core (pure dense FFN pipeline at the PE roofline):
  for each expert chunk (all 8 experts, capacity C_e, SUM C_e ~ 2054):
    hT[h, j]  = gelu(W1[e].T @ xselT + b1[e])   (f16, ACT does gelu+bias)
    eoT[d, j] = W2[e].T-tiles @ hT              (transposed layer 2: cost
                                                 is C cycles/matmul, no
                                                 partial-tile waste)
    DMA eoT chunk to DRAM (raw, un-gated; host applies b2/gates).
  Weights stream fine-grained (host pre-swizzled for contiguous DMA):
  W1 as 32 ht-column slices of 0.25MB on the sync queue, W2 as 8 d-column
  slices of 1MB on the gpsimd queue, consumed in arrival order with deep
  buffer pools -> no monolithic transfer ever head-of-line blocks a
  latency-critical load, and out stores ride the scalar queue.
"""

import os
import sys

for _p in ("/root/.axon_site/_ro/trn_rl_repo", "/opt/trn_rl_repo"):
    if os.path.isdir(_p) and _p not in sys.path:
        sys.path.insert(0, _p)

import numpy as np
import ml_dtypes

import concourse.bass as bass  # noqa: F401  (kept for parity with utils)
import concourse.bacc as bacc
import concourse.tile as tile
from concourse import mybir
from concourse.bass_utils import run_bass_kernel_spmd

F32 = mybir.dt.float32
F16 = mybir.dt.float16
BF16 = mybir.dt.bfloat16
AF = mybir.ActivationFunctionType

D = 1024      # in_features
H = 4096      # hidden
E = 8         # experts
TOPK = 2
N_CORES = 8
N = 8192      # total tokens
ND = D // 128   # 8 feature tiles
NH = H // 128   # 32 hidden tiles

REPS = 1   # device-side repeat loop (timing only; >1 wraps body in For_i)
SKIP = set()   # timing ablations: subsets of {"dma", "mm", "act", "out"}


# --------------------------------------------------------------------------
# host routing + balanced assignment
# --------------------------------------------------------------------------

def _route(x, Wg, bg):
    """f64 router: top-2 expert ids (stable tie-break, like lax.top_k) and
    softmax gate weights."""
    xt = np.asarray(x, np.float64).reshape(-1, D)
    logits = xt @ np.asarray(Wg, np.float64) + np.asarray(bg, np.float64)
    top2 = np.argsort(-logits, axis=1, kind="stable")[:, :TOPK]
    m = logits.max(axis=1, keepdims=True)
    p = np.exp(logits - m)
    p /= p.sum(axis=1, keepdims=True)
    gates = np.take_along_axis(p, top2, axis=1)
    return top2, gates.astype(np.float32)


def _balance(top2):
    """Greedy token->core assignment: 1024 tokens/core, minimizing the max
    per-core count of each token's two experts. Gets per-expert counts
    within ~2 of N_e/8 on every core."""
    cnt = np.zeros((N_CORES, E), np.int64)
    ntok = np.zeros(N_CORES, np.int64)
    core_of = np.full(N, -1, np.int64)
    pair_key = top2[:, 0] * E + top2[:, 1]
    order = np.argsort(pair_key, kind="stable")
    big = 1 << 30
    cap = N // N_CORES
    for t in order:
        e1, e2 = top2[t]
        load = cnt[:, e1] + cnt[:, e2] + (ntok >= cap) * big
        c = int(np.argmin(load))
        core_of[t] = c
        cnt[c, e1] += 1
        cnt[c, e2] += 1
        ntok[c] += 1
    return core_of, cnt


class _Plan:
    __slots__ = ("top2", "gates", "core_of", "caps", "offs", "SC",
                 "slot_tok", "colof", "_pin")


_PLAN_CACHE: dict = {}


def _plan(x, Wg, bg):
    key = (id(x), id(Wg))
    hit = _PLAN_CACHE.get(key)
    if hit is not None:
        return hit
    top2, gates = _route(x, Wg, bg)
    core_of, cnt = _balance(top2)
    caps = tuple(int(v) for v in cnt.max(axis=0))
    offs = np.concatenate([[0], np.cumsum(caps)[:-1]]).astype(np.int64)
    SC = int(sum(caps))
    slot_tok = np.zeros((N_CORES, SC), np.int64)
    colof = np.zeros((N, TOPK), np.int64)
    for c in range(N_CORES):
        tl = np.where(core_of == c)[0]
        pairs = top2[tl]
        for e in range(E):
            sel = tl[(pairs == e).any(axis=1)]
            k = len(sel)
            slot_tok[c, offs[e]:offs[e] + k] = sel
            r = np.where(top2[sel, 0] == e, 0, 1)
            colof[sel, r] = c * SC + offs[e] + np.arange(k)
    p = _Plan()
    p.top2, p.gates, p.core_of, p.caps, p.offs, p.SC = (
        top2, gates, core_of, caps, offs, SC)
    p.slot_tok, p.colof = slot_tok, colof
    p._pin = (x, Wg)   # keep ids alive for the cache key
    _PLAN_CACHE.clear()
    _PLAN_CACHE[key] = p
    return p


def route_capacities(inputs):
    """Compile-time shapes: per-expert capacities (uniform across cores)."""
    return _plan(inputs["x"], inputs["Wg"], inputs["bg"]).caps


# --------------------------------------------------------------------------
# device program
# --------------------------------------------------------------------------

def _emit(nc, tc, io, caps):
    from contextlib import ExitStack

    with ExitStack() as ctx:
        cpool = ctx.enter_context(tc.tile_pool(name="const", bufs=1))
        xpool = ctx.enter_context(tc.tile_pool(name="xs", bufs=3))
        w1pool = ctx.enter_context(tc.tile_pool(name="w1", bufs=36))
        w2pool = ctx.enter_context(tc.tile_pool(name="w2", bufs=8))
        hpool = ctx.enter_context(tc.tile_pool(name="hT", bufs=2))
        opool = ctx.enter_context(tc.tile_pool(name="ot", bufs=2))
        ps1 = ctx.enter_context(tc.tile_pool(name="ps1", bufs=4, space="PSUM"))
        ps2 = ctx.enter_context(tc.tile_pool(name="ps2", bufs=4, space="PSUM"))

        DMA = "dma" not in SKIP
        MM = "mm" not in SKIP
        ACT = "act" not in SKIP
        OUT = "out" not in SKIP
        L1 = "l1" not in SKIP
        L2 = "l2" not in SKIP

        b1c = cpool.tile([128, E * NH], F32)
        nc.scalar.dma_start(b1c[:], io["b1c"].ap())

        W1d = io["W1c"].ap()       # [E, NH, 128, ND*128] f16 (pre-swizzled)
        W2d = io["W2s"].ap()       # [E, ND, 128, H] f16 (pre-swizzled)
        Xd = io["xseltc"].ap()     # [128, ND*SC] f16
        Od = io["out"].ap()        # [128, ND*SC] bf16

        offs = np.concatenate([[0], np.cumsum(caps)[:-1]]).astype(int)

        def load_xs(w):
            C, off = caps[w], offs[w]
            xs = xpool.tile([128, ND * C], F16, tag="xs", name=f"xs{w}")
            if DMA:
                nc.gpsimd.dma_start(xs[:], Xd[:, ND * off: ND * off + ND * C])
            return xs

        # xs for chunk w is requested one chunk ahead
        xs_next = load_xs(0)
        for w, C in enumerate(caps):
            off = offs[w]
            xs = xs_next
            if w + 1 < len(caps):
                xs_next = load_xs(w + 1)

            # ---- layer 1: hT[h, j] = gelu(W1.T @ xselT + b1) ----
            # W1 streams as 32 ht-column slices (0.25 MB each, sync queue),
            # consumed in arrival order -> no monolithic-transfer stalls.
            hT = hpool.tile([128, NH * C], F16, tag="hT")
            if not ACT or not L1:
                nc.vector.memset(hT[:], 0.5)
            for ht in range(NH):
                if not L1:
                    break
                w1c = w1pool.tile([128, ND * 128], F16, tag="w1c")
                if DMA:
                    nc.sync.dma_start(w1c[:], W1d[w, ht])
                psf = ps1.tile([128, 512], F32, tag="ps1", name="psf1")
                ps = psf[:, :C]
                if MM:
                    for dt in range(ND):
                        nc.tensor.matmul(
                            ps[:],
                            lhsT=w1c[:, dt * 128:(dt + 1) * 128],
                            rhs=xs[:, dt * C:(dt + 1) * C],
                            start=(dt == 0),
                            stop=(dt == ND - 1),
                        )
                if ACT and MM and L1:
                    nc.scalar.activation(
                        hT[:, ht * C:(ht + 1) * C],
                        ps[:],
                        AF.Gelu,
                        bias=b1c[:, w * NH + ht: w * NH + ht + 1],
                    )

            # ---- layer 2 (transposed): eoT[d, j] = W2-col-tiles.T @ hT ----
            ot = opool.tile([128, ND * C], BF16, tag="ot")
            for dt in range(ND):
                if not L2:
                    break
                w2 = w2pool.tile([128, H], F16, tag="w2")
                if DMA:
                    nc.gpsimd.dma_start(w2[:], W2d[w, dt])
                psf = ps2.tile([128, 512], F32, tag="ps2", name="psf2")
                ps = psf[:, :C]
                if MM:
                    for ht in range(NH):
                        nc.tensor.matmul(
                            ps[:],
                            lhsT=w2[:, ht * 128:(ht + 1) * 128],
                            rhs=hT[:, ht * C:(ht + 1) * C],
                            start=(ht == 0),
                            stop=(ht == NH - 1),
                        )
                if ACT and MM:
                    nc.scalar.copy(ot[:, dt * C:(dt + 1) * C], ps[:])
            if OUT and ACT and MM and L2:
                nc.scalar.dma_start(Od[:, ND * off: ND * off + ND * C], ot[:])


def _build_sparse(caps):
    nc = bacc.Bacc(None, target_bir_lowering=False, debug=False,
                   num_devices=N_CORES)
    SC = int(sum(caps))
    io = {
        "xseltc": nc.declare_dram_parameter("xseltc", [128, ND * SC], F16,
                                            isOutput=False),
        "W1c": nc.declare_dram_parameter("W1c", [E, NH, 128, ND * 128], F16,
                                         isOutput=False),
        "W2s": nc.declare_dram_parameter("W2s", [E, ND, 128, H], F16,
                                         isOutput=False),
        "b1c": nc.declare_dram_parameter("b1c", [128, E * NH], F32,
                                         isOutput=False),
        "out": nc.declare_dram_parameter("out", [128, ND * SC], BF16,
                                         isOutput=True),
    }
    with tile.TileContext(nc) as tc:
        if REPS > 1:
            with tc.For_i(0, REPS, 1):
                _emit(nc, tc, io, caps)
        else:
            _emit(nc, tc, io, caps)
    nc.compile()
    return nc


# --------------------------------------------------------------------------
# host prep / combine
# --------------------------------------------------------------------------

def prep_inputs(x, Wg, bg, W1, b1, W2, b2):
    """Host-side shard + layout/dtype prep. Returns per-core input maps."""
    plan = _plan(x, Wg, bg)
    caps, offs, SC = plan.caps, plan.offs, plan.SC

    xt = np.asarray(x, np.float32).reshape(-1, D)
    W1h = np.asarray(W1, np.float32).astype(np.float16)          # [E, D, H]
    # W1c[e, ht, p, dt*128+j] = W1[e, dt*128+p, ht*128+j]
    W1c = np.ascontiguousarray(
        W1h.reshape(E, ND, 128, NH, 128).transpose(0, 3, 2, 1, 4)
    ).reshape(E, NH, 128, ND * 128)
    W2h = np.asarray(W2, np.float32).astype(np.float16)          # [E, H, D]
    # W2s[e, dt, p, ht*128+m] = W2[e, ht*128+p, dt*128+m]
    W2s = np.ascontiguousarray(
        W2h.reshape(E, NH, 128, ND, 128).transpose(0, 3, 2, 1, 4)
    ).reshape(E, ND, 128, H)
    b1c = np.ascontiguousarray(
        np.asarray(b1, np.float32).reshape(E, NH, 128).transpose(2, 0, 1)
    ).reshape(128, E * NH)

    in_maps = []
    for c in range(N_CORES):
        xseltc = np.empty((128, ND * SC), np.float16)
        for e in range(E):
            C = caps[e]
            cols = plan.slot_tok[c, offs[e]:offs[e] + C]
            xsel = xt[cols]                                  # [C, D]
            blk = xsel.T.reshape(ND, 128, C).transpose(1, 0, 2)
            xseltc[:, ND * offs[e]: ND * offs[e] + ND * C] = (
                blk.reshape(128, ND * C))
        in_maps.append({
            "xseltc": xseltc,
            "W1c": W1c,
            "W2s": W2s,
            "b1c": b1c,
        })
    return in_maps


_CACHE = {}


def kernel(x, Wg, bg, W1, b1, W2, b2):
    B_, S_, D_ = x.shape
    plan = _plan(x, Wg, bg)
    caps, offs, SC = plan.caps, plan.offs, plan.SC
    in_maps = prep_inputs(x, Wg, bg, W1, b1, W2, b2)

    key = ("v2", caps)
    if key not in _CACHE:
        _CACHE[key] = _build_sparse(caps)
    nc = _CACHE[key]
    res = run_bass_kernel_spmd(nc, in_maps, list(range(N_CORES)))

    # decode per-core outputs into eo_all[d, global_slot]
    eo_all = np.empty((D, N_CORES * SC), np.float32)
    for c in range(N_CORES):
        oc = np.asarray(res.results[c]["out"]).astype(np.float32)
        for e in range(E):
            C = caps[e]
            blk = oc[:, ND * offs[e]: ND * offs[e] + ND * C]
            eo_all[:, c * SC + offs[e]: c * SC + offs[e] + C] = (
                blk.reshape(128, ND, C).transpose(1, 0, 2).reshape(D, C))

    # combine: out[t] = sum_k g_k * (eo[:, col_k] + b2[e_k])
    b2f = np.asarray(b2, np.float32)
    g = plan.gates
    t2 = plan.top2
    out = g[:, 0:1] * (eo_all[:, plan.colof[:, 0]].T + b2f[t2[:, 0]])
    out += g[:, 1:2] * (eo_all[:, plan.colof[:, 1]].T + b2f[t2[:, 1]])
    return out.reshape(B_, S_, D_).astype(np.float32)


if __name__ == "__main__":
    sys.path.insert(0, "/root/problem")
    npz = "/root/problem/_inputs.npz"
    if os.path.exists(npz):
        dat = np.load(npz)
        inputs = {k: dat[k] for k in ("x", "Wg", "bg", "W1", "b1", "W2", "b2")}
        want = dat["ref"]
    else:
        os.environ.setdefault("JAX_PLATFORMS", "")
        import reference as R
        inputs = {k: np.asarray(v) for k, v in R.setup_inputs().items()}
        want = None

    got = kernel(**inputs)
    if want is not None:
        diff = np.abs(got - want)
        scale = np.abs(want).max()
        rel_fro = np.linalg.norm(diff) / np.linalg.norm(want)
        print(f"absmax err: {diff.max():.3e}  scale: {scale:.3e}  "
              f"absmax/scale: {diff.max() / scale:.3e}  rel_fro: {rel_fro:.3e}")
